# revision 17
# baseline (speedup 1.0000x reference)
"""BlockDecay (RetNet-style chunkwise linear attention with per-feature decay)
Trainium2 Bass kernel, batch-parallel over 8 NeuronCores.

Math (per batch): out[t] = sum_r q[t,r] * S_t[r,:],
  S_t[r,d] = sum_{s<=t} gamma_r^{t-s} k[s,r] h[s,d]
computed chunkwise with C=256 using the standard factorization
  A[i,j] = (q gamma^i) . (k gamma^-j),  intra = (A*mask) @ h,
  inter  = (q gamma^{i+1}) @ S_chunk,   S' = gamma^{256} S' + K',
  K'[r,d] = sum_j gamma_r^{256-j} k[j,r] h[j,d]   (S' = gamma*S folded scale)

Device inputs (host pre-scales/transposes; k is shipped ONCE as k2n —
ksT is derived on-device: ksT[r,j] = transpose(k2n)[r,j] * gamma_r^-256):
  qsT [R, W] = (q * gamma^(i%C)).T
  k2n [128, W]  block-local [j, (blk, r)] = k*gamma^(C - j%C)
  hn  [128, W]  block-local [j, (blk, d)]
  tri [128, 128] causal mask transposed (i>=j pattern)
  g2  [128, 2] = [gamma^256 | gamma^-256]
  id128 [128, 128] identity (PE transpose operand)
Output otT [D, W] (transposed), host transposes back.
"""
import os
import sys
import numpy as np

for _p in ("/root/.axon_site", "/root/.axon_site/_ro/trn_rl_repo",
           "/root/.axon_site/_ro/pypackages"):
    if _p not in sys.path and os.path.isdir(_p):
        sys.path.append(_p)

B, W, R, D = 8, 4096, 128, 128
C = 256
NCH = W // C
NBLK = W // 128

_PROG = {}


def _patched_tc(nc):
    """TileContext with a cheap exit: per-sem drains distributed across
    engines (this walrus accepts one sync-wait per instruction), one
    barrier, then sem clears for idempotent re-execution.  The final join
    is walrus's own BSP model-end sync."""
    import concourse.tile as tile
    import concourse.tile_sem_assignment as tsa
    from concourse.tile import ScopedClock

    class PatchedTileContext(tile.TileContext):
        def _drain_and_barrier(self, tick_clock, wait_clock):
            gc = tick_clock.global_clock
            n = tsa.N_PROCS
            nc = self.nc
            # all drains on sync: its stream naturally ends last, and a
            # blocking drain on an early-finishing engine (esp. gpsimd)
            # stalls SWDGE descriptor processing mid-kernel
            for p in range(n):
                ticks = gc[p]
                if ticks <= 0:
                    continue
                d = nc.sync.drain()
                wait_clock.add_sem_waits(
                    d.ins,
                    ScopedClock({None: tsa.VectorClock(
                        [ticks if q == p else 0 for q in range(n)])}),
                )
            nc.all_engine_barrier()
            assert self.sems is not None
            popped = nc._tile_sem_poison_stack.pop()
            assert popped is self._sem_poison
            nc.clear_and_free_semaphores(list(self.sems.allocated().values()))

    return PatchedTileContext(nc)


def _split_multi_waits(nc, limit=1):
    """Hoist extra sync-waits onto injected same-engine NoOps (in-order
    engines make waiting earlier in the stream safe)."""
    import concourse.mybir as mybir
    n_new = 0
    for fn in nc.m.functions:
        for bb in fn.blocks:
            out = []
            changed = False
            for inst in bb.instructions:
                si = getattr(inst, "sync_info", None)
                waits = list(si.on_wait) if si is not None and si.on_wait else []
                if len(waits) > limit:
                    for w in waits[:-limit]:
                        nop = mybir.InstNoOp(
                            name=f"I-wsplit-{n_new}",
                            engine=inst.engine,
                            sync_info=mybir.SyncInfo(on_wait=[w], on_update=[]),
                        )
                        n_new += 1
                        out.append(nop)
                    si.on_wait = waits[-limit:]
                    changed = True
                out.append(inst)
            if changed:
                bb.instructions = out
    return n_new


def _blk(n2):
    return slice(n2 * 128, (n2 + 1) * 128)


def _build_program(use_f32r=True):
    key = ("v6", use_f32r)
    if key in _PROG:
        return _PROG[key]
    import concourse.bass as bass
    import concourse.mybir as mybir

    F32 = mybir.dt.float32
    FM = mybir.dt.float32r if use_f32r else F32

    nc = bass.Bass()
    qsT = nc.declare_dram_parameter("qsT", [128, W], FM, isOutput=False)
    k2n = nc.declare_dram_parameter("k2n", [128, W], FM, isOutput=False)
    hn = nc.declare_dram_parameter("hn", [128, W], FM, isOutput=False)
    tri = nc.declare_dram_parameter("tri", [128, 128], F32, isOutput=False)
    g2 = nc.declare_dram_parameter("g2", [128, 2], F32, isOutput=False)
    id128 = nc.declare_dram_parameter("id128", [128, 128], FM, isOutput=False)
    otT = nc.declare_dram_parameter("otT", [128, W], F32, isOutput=True)

    mm = nc.tensor.matmul
    with _patched_tc(nc) as tc:
        with tc.tile_pool(name="big", bufs=1) as big, \
             tc.tile_pool(name="small", bufs=1) as small, \
             tc.tile_pool(name="st", bufs=4) as stp, \
             tc.tile_pool(name="am0p", bufs=3) as am0p, \
             tc.tile_pool(name="am1p", bufs=3) as am1p, \
             tc.tile_pool(name="ps_at", bufs=2, space="PSUM") as ps_at, \
             tc.tile_pool(name="ps_ot", bufs=2, space="PSUM") as ps_ot, \
             tc.tile_pool(name="ps_kp", bufs=2, space="PSUM") as ps_kp, \
             tc.tile_pool(name="ps_tr", bufs=2, space="PSUM") as ps_tr:

            qsT_sb = big.tile([128, W], FM, tag="qsT")
            ksT_sb = big.tile([128, W], FM, tag="ksT")   # derived on-device
            k2n_sb = big.tile([128, W], FM, tag="k2n")
            hn_sb = big.tile([128, W], FM, tag="hn")
            otT_sb = big.tile([128, W], F32, tag="otT")
            tri_sb = small.tile([128, 128], F32, tag="tri")
            g2_sb = small.tile([128, 2], F32, tag="g2")
            id_sb = small.tile([128, 128], FM, tag="id128")

            # PE warm-up: dummy fp32 matmuls fill the DMA-wait window and
            # flip the HAM clock gate to 8/8 before the real stream starts.
            wz = small.tile([128, 256], F32, tag="wz")
            nc.vector.memset(wz[:], 0.0)
            for _ in range(6):
                wp = ps_ot.tile([128, 256], F32, tag="ot")
                mm(wp[:], wz[:, :128], wz[:], start=True, stop=True)

            # bulk inputs on the two HWDGE engines (fast RTL descgen).  Only
            # 8 HWDGE sem lanes exist globally and an issue blocks its
            # sequencer on lane reuse, so the late (reuse-waiting) issues all
            # go on sync, which has no compute; scalar/ACT gets only the
            # first-round issues.  Outputs go on gpsimd/SWDGE: its own lane
            # pool, and gpsimd is otherwise idle.
            # First 8 global HWDGE issues carry the critical small pieces +
            # consts (no lane-reuse wait can block them); late big pieces go
            # via gpsimd/SWDGE which has its own lane pool; outputs go on
            # sync (idle by then).
            nc.sync.dma_start(k2n_sb[:, 0:256], k2n[:, 0:256])
            nc.scalar.dma_start(hn_sb[:, 0:256], hn[:, 0:256])
            nc.sync.dma_start(qsT_sb[:, 0:256], qsT[:, 0:256])
            nc.scalar.dma_start(g2_sb[:], g2[:])
            nc.sync.dma_start(k2n_sb[:, 256:1280], k2n[:, 256:1280])
            nc.scalar.dma_start(id_sb[:], id128[:])
            nc.sync.dma_start(qsT_sb[:, 256:1280], qsT[:, 256:1280])
            nc.scalar.dma_start(hn_sb[:, 256:1280], hn[:, 256:1280])
            nc.sync.dma_start(tri_sb[:], tri[:])
            nc.sync.dma_start(k2n_sb[:, 1280:2432], k2n[:, 1280:2432])
            nc.scalar.dma_start(qsT_sb[:, 1280:2432], qsT[:, 1280:2432])
            nc.gpsimd.dma_start(hn_sb[:, 1280:2432], hn[:, 1280:2432])
            nc.gpsimd.dma_start(k2n_sb[:, 2432:4096], k2n[:, 2432:4096])
            nc.gpsimd.dma_start(qsT_sb[:, 2432:4096], qsT[:, 2432:4096])
            nc.gpsimd.dma_start(hn_sb[:, 2432:4096], hn[:, 2432:4096])

            S_prev = stp.tile([128, 128], FM, tag="S")
            if use_f32r:
                nc.vector.tensor_scalar_mul(S_prev[:], wz[:, :128], 0.0)
            else:
                nc.vector.memset(S_prev[:], 0.0)


            def derive_ksT(mx):
                """ksT[:, chunk mx] = transpose(k2n blocks) * gamma^-256;
                the per-partition rescale alternates DVE/ACT for balance."""
                for b2 in (2 * mx, 2 * mx + 1):
                    TR = ps_tr.tile([128, 128], FM, tag="tr")
                    nc.tensor.transpose(TR[:], k2n_sb[:, _blk(b2)], id_sb[:])
                    if b2 % 2 == 0:
                        nc.vector.tensor_scalar_mul(
                            ksT_sb[:, _blk(b2)], TR[:], g2_sb[:, 1:2])
                    else:
                        nc.scalar.mul(ksT_sb[:, _blk(b2)], TR[:],
                                      g2_sb[:, 1:2])

            derive_ksT(0)
            pend = None
            for m in range(NCH):
                c = m * C
                j0 = slice(c, c + 128)
                j1 = slice(c + 128, c + 256)
                ci = slice(c, c + 256)
                ch = slice(c + 128, c + 256)

                KP = ps_kp.tile([128, 128], F32, tag="kp")
                mm(KP[:], k2n_sb[:, j0], hn_sb[:, j0], start=True, stop=False)
                mm(KP[:], k2n_sb[:, j1], hn_sb[:, j1], start=False, stop=True)
                S_new = stp.tile([128, 128], FM, tag="S")
                nc.vector.scalar_tensor_tensor(
                    out=S_new[:], in0=S_prev[:], scalar=g2_sb[:, 0:1],
                    in1=KP[:], op0=mybir.AluOpType.mult, op1=mybir.AluOpType.add)

                if m + 1 < NCH:
                    derive_ksT(m + 1)

                ATb = ps_at.tile([128, 512], F32, tag="at")
                mm(ATb[:, 0:256], ksT_sb[:, j0], qsT_sb[:, ci],
                   start=True, stop=True)
                Am0 = am0p.tile([128, 256], FM, tag="am0")
                nc.vector.tensor_mul(Am0[:, 0:128], ATb[:, 0:128], tri_sb[:])
                nc.scalar.copy(Am0[:, 128:256], ATb[:, 128:256])
                mm(ATb[:, 256:384], ksT_sb[:, j1], qsT_sb[:, ch],
                   start=True, stop=True)
                Am1 = am1p.tile([128, 128], FM, tag="am1")
                nc.vector.tensor_mul(Am1[:], ATb[:, 256:384], tri_sb[:])

                if pend is not None:
                    _emit_out(nc, mm, pend, use_f32r, hn_sb, qsT_sb, otT_sb,
                              otT, ps_ot)
                pend = (m, S_prev, Am0, Am1)
                S_prev = S_new
            _emit_out(nc, mm, pend, use_f32r, hn_sb, qsT_sb, otT_sb, otT, ps_ot)

    _split_multi_waits(nc)
    _PROG[key] = nc
    return nc


_OUT_PIECES = {3: (0, 4), 7: (4, 8), 11: (8, 12), 13: (12, 14),
               14: (14, 15), 15: (15, 16)}


def _emit_out(nc, mm, pend, use_f32r, hn_sb, qsT_sb, otT_sb, otT, ps_ot):
    import concourse.mybir as mybir
    m, S_m, Am0, Am1 = pend
    c = m * C
    j0 = slice(c, c + 128)
    j1 = slice(c + 128, c + 256)
    ci = slice(c, c + 256)
    OT = ps_ot.tile([128, 256], mybir.dt.float32, tag="ot")
    mm(OT[:], hn_sb[:, j0], Am0[:], start=True, stop=False)
    mm(OT[:, 128:256], hn_sb[:, j1], Am1[:], start=False, stop=False)
    mm(OT[:], S_m[:], qsT_sb[:, ci], start=False, stop=True)
    nc.scalar.copy(otT_sb[:, ci], OT[:])
    if m in _OUT_PIECES:
        lo, hi = _OUT_PIECES[m]
        s = slice(lo * C, hi * C)
        nc.sync.dma_start(otT[:, s], otT_sb[:, s])


def _host_prep(q_alpha, k, h_norm, gamma_vec, causal_mask):
    gamma = np.clip(gamma_vec.astype(np.float64), 1e-8, None)
    log_g = np.log(gamma)
    i_loc = (np.arange(W) % C).astype(np.float64)
    Sq = np.exp(np.outer(i_loc, log_g))          # [W, R] gamma^(i%C)
    Sk2 = np.exp(np.outer(C - i_loc, log_g))     # gamma^(C - j%C)
    g2 = np.stack([np.exp(C * log_g), np.exp(-C * log_g)],
                  axis=1).astype(np.float32)     # [128, 2]

    tri = np.ascontiguousarray(causal_mask.T, np.float32)
    id128 = np.eye(128, dtype=np.float32)

    def blockify(x):  # [W, 128] -> [128, (blk, 128)]
        return np.ascontiguousarray(
            x.reshape(NBLK, 128, 128).transpose(1, 0, 2).reshape(128, W))

    in_maps = []
    for b in range(B):
        q64 = q_alpha[b].astype(np.float64)
        k64 = k[b].astype(np.float64)
        in_maps.append({
            "qsT": np.ascontiguousarray((q64 * Sq).T.astype(np.float32)),
            "k2n": blockify((k64 * Sk2).astype(np.float32)),
            "hn": blockify(np.ascontiguousarray(h_norm[b], np.float32)),
            "tri": tri,
            "g2": g2,
            "id128": id128,
        })
    return in_maps


def _ensure_ntff_hook():
    try:
        from antenv import axon_hooks  # noqa: F401
        return
    except ImportError:
        pass
    import types
    import antenv
    try:
        import trn_agent_boot.trn_boot as tb
        hook = tb._ntff_profile_via_ctypes("/opt/axon/libaxon_pjrt.so")
    except Exception:
        hook = None
    mod = types.ModuleType("antenv.axon_hooks")
    mod.get_axon_ntff_profile_hook = lambda: hook
    mod.set_axon_ntff_profile_hook = lambda h: None
    sys.modules["antenv.axon_hooks"] = mod
    antenv.axon_hooks = mod


_last = {"exec_time_ns": None}


def kernel(q_alpha, k, h_norm, gamma_vec, causal_mask, decay_diff,
           _trace=False, _use_f32r=None):
    if _use_f32r is None:
        _use_f32r = os.environ.get("BD_F32R", "1") == "1"
    trace = _trace or os.environ.get("BD_TRACE", "0") == "1"
    from concourse.bass_utils import run_bass_kernel_spmd

    nc = _build_program(use_f32r=_use_f32r)
    in_maps = _host_prep(q_alpha, k, h_norm, gamma_vec, causal_mask)
    kwargs = {}
    if trace:
        _ensure_ntff_hook()
        import concourse.bass_utils as bu
        bu.upload_artifacts = lambda tmpdir: tmpdir  # no bucket in container
        kwargs = dict(trace=True, tmpdir=os.environ.get("BD_TRACE_DIR") or None)
    res = run_bass_kernel_spmd(nc, in_maps, list(range(B)), **kwargs)
    _last["exec_time_ns"] = res.exec_time_ns
    out = np.empty((B, W, D), np.float32)
    for b in range(B):
        out[b] = res.results[b]["otT"].T
    return out


# revision 18
# speedup vs baseline: 1.1007x; 1.1007x over previous
"""BlockDecay (RetNet-style chunkwise linear attention with per-feature decay)
Trainium2 Bass kernel, batch-parallel over 8 NeuronCores.

Math (per batch): out[t] = sum_r q[t,r] * S_t[r,:],
  S_t[r,d] = sum_{s<=t} gamma_r^{t-s} k[s,r] h[s,d]
computed chunkwise with C=256 using the standard factorization
  A[i,j] = (q gamma^i) . (k gamma^-j),  intra = (A*mask) @ h,
  inter  = (q gamma^{i+1}) @ S_chunk,   S' = gamma^{256} S' + K',
  K'[r,d] = sum_j gamma_r^{256-j} k[j,r] h[j,d]   (S' = gamma*S folded scale)

Host pre-scales/transposes all operands; device layout:
  qsT [R, W] = (q * gamma^(i%C)).T
  ksT [R, W] = (k * gamma^-(j%C)).T
  k2n [128, W]  block-local [j, (blk, r)] = k*gamma^(C - j%C)
  hn  [128, W]  block-local [j, (blk, d)]
  tri [128, 128] causal mask transposed (i>=j pattern)
  g256 [128, 1] = gamma^256
Output otT [D, W] (transposed), host transposes back.

BD_F32R=1 switches the matmuls to float32r (fp32_mode=HIGH single-pass,
2 cyc/row instead of fp32's 4): faster, but output error grows from
~5e-7 to ~2.6e-4 (relative to out absmax).  Default is full fp32.
"""
import os
import sys
import numpy as np

for _p in ("/root/.axon_site", "/root/.axon_site/_ro/trn_rl_repo",
           "/root/.axon_site/_ro/pypackages"):
    if _p not in sys.path and os.path.isdir(_p):
        sys.path.append(_p)

B, W, R, D = 8, 4096, 128, 128
C = 256
NCH = W // C
NBLK = W // 128

_PROG = {}


def _patched_tc(nc):
    """TileContext with a cheap exit: per-sem single-wait drains on sync
    (this walrus accepts one sync-wait per instruction, and a blocking
    drain on an early-finishing engine stalls SWDGE descriptor handling),
    one barrier, then sem clears for idempotent re-execution.  The final
    join is walrus's own BSP model-end sync."""
    import concourse.tile as tile
    import concourse.tile_sem_assignment as tsa
    from concourse.tile import ScopedClock

    class PatchedTileContext(tile.TileContext):
        def _drain_and_barrier(self, tick_clock, wait_clock):
            gc = tick_clock.global_clock
            n = tsa.N_PROCS
            nc = self.nc
            for p in range(n):
                ticks = gc[p]
                if ticks <= 0:
                    continue
                d = nc.sync.drain()
                wait_clock.add_sem_waits(
                    d.ins,
                    ScopedClock({None: tsa.VectorClock(
                        [ticks if q == p else 0 for q in range(n)])}),
                )
            nc.all_engine_barrier()
            assert self.sems is not None
            popped = nc._tile_sem_poison_stack.pop()
            assert popped is self._sem_poison
            nc.clear_and_free_semaphores(list(self.sems.allocated().values()))

    return PatchedTileContext(nc)


def _split_multi_waits(nc, limit=1):
    """Hoist extra sync-waits onto injected same-engine NoOps (in-order
    engines make waiting earlier in the stream safe)."""
    import concourse.mybir as mybir
    n_new = 0
    for fn in nc.m.functions:
        for bb in fn.blocks:
            out = []
            changed = False
            for inst in bb.instructions:
                si = getattr(inst, "sync_info", None)
                waits = list(si.on_wait) if si is not None and si.on_wait else []
                if len(waits) > limit:
                    for w in waits[:-limit]:
                        nop = mybir.InstNoOp(
                            name=f"I-wsplit-{n_new}",
                            engine=inst.engine,
                            sync_info=mybir.SyncInfo(on_wait=[w], on_update=[]),
                        )
                        n_new += 1
                        out.append(nop)
                    si.on_wait = waits[-limit:]
                    changed = True
                out.append(inst)
            if changed:
                bb.instructions = out
    return n_new


def _build_program(use_f32r):
    key = ("v7", use_f32r)
    if key in _PROG:
        return _PROG[key]
    import concourse.bass as bass
    import concourse.mybir as mybir

    F32 = mybir.dt.float32
    FM = mybir.dt.float32r if use_f32r else F32

    nc = bass.Bass()
    qsT = nc.declare_dram_parameter("qsT", [128, W], FM, isOutput=False)
    ksT = nc.declare_dram_parameter("ksT", [128, W], FM, isOutput=False)
    k2n = nc.declare_dram_parameter("k2n", [128, W], FM, isOutput=False)
    hn = nc.declare_dram_parameter("hn", [128, W], FM, isOutput=False)
    tri = nc.declare_dram_parameter("tri", [128, 128], F32, isOutput=False)
    g256 = nc.declare_dram_parameter("g256", [128, 1], F32, isOutput=False)
    otT = nc.declare_dram_parameter("otT", [128, W], F32, isOutput=True)

    mm = nc.tensor.matmul
    with _patched_tc(nc) as tc:
        with tc.tile_pool(name="big", bufs=1) as big, \
             tc.tile_pool(name="small", bufs=1) as small, \
             tc.tile_pool(name="st", bufs=4) as stp, \
             tc.tile_pool(name="am0p", bufs=3) as am0p, \
             tc.tile_pool(name="am1p", bufs=3) as am1p, \
             tc.tile_pool(name="ps_at", bufs=2, space="PSUM") as ps_at, \
             tc.tile_pool(name="ps_ot", bufs=2, space="PSUM") as ps_ot, \
             tc.tile_pool(name="ps_kp", bufs=2, space="PSUM") as ps_kp:

            qsT_sb = big.tile([128, W], FM, tag="qsT")
            ksT_sb = big.tile([128, W], FM, tag="ksT")
            k2n_sb = big.tile([128, W], FM, tag="k2n")
            hn_sb = big.tile([128, W], FM, tag="hn")
            otT_sb = big.tile([128, W], F32, tag="otT")
            tri_sb = small.tile([128, 128], F32, tag="tri")
            g256_sb = small.tile([128, 1], F32, tag="g256")

            # PE warm-up: dummy fp32 matmuls fill the DMA-wait window and
            # flip the HAM clock gate to 8/8 before the real stream starts.
            wz = small.tile([128, 256], F32, tag="wz")
            nc.vector.memset(wz[:], 0.0)
            for _ in range(6):
                wp = ps_ot.tile([128, 256], F32, tag="ot")
                mm(wp[:], wz[:, :128], wz[:], start=True, stop=True)

            # inputs split across the HWDGE ring (sync) and the SWDGE ring
            # (gpsimd, which issues nothing else afterwards); consts +
            # outputs ride on scalar/HWDGE
            nc.scalar.dma_start(tri_sb[:], tri[:])
            nc.scalar.dma_start(g256_sb[:], g256[:])
            P = W // 4
            for p in range(4):
                s = slice(p * P, (p + 1) * P)
                nc.sync.dma_start(k2n_sb[:, s], k2n[:, s])
                nc.gpsimd.dma_start(hn_sb[:, s], hn[:, s])
                nc.sync.dma_start(ksT_sb[:, s], ksT[:, s])
                nc.gpsimd.dma_start(qsT_sb[:, s], qsT[:, s])

            S_prev = stp.tile([128, 128], FM, tag="S")
            if use_f32r:
                nc.vector.tensor_scalar_mul(S_prev[:], wz[:, :128], 0.0)
            else:
                nc.vector.memset(S_prev[:], 0.0)

            pend = None
            for m in range(NCH):
                c = m * C
                j0 = slice(c, c + 128)
                j1 = slice(c + 128, c + 256)
                ci = slice(c, c + 256)
                ch = slice(c + 128, c + 256)

                KP = ps_kp.tile([128, 128], F32, tag="kp")
                mm(KP[:], k2n_sb[:, j0], hn_sb[:, j0], start=True, stop=False)
                mm(KP[:], k2n_sb[:, j1], hn_sb[:, j1], start=False, stop=True)
                S_new = stp.tile([128, 128], FM, tag="S")
                nc.vector.scalar_tensor_tensor(
                    out=S_new[:], in0=S_prev[:], scalar=g256_sb[:, 0:1],
                    in1=KP[:], op0=mybir.AluOpType.mult, op1=mybir.AluOpType.add)

                # AT0 full [j0 x 256i]; AT1 only needs i in [128,256)
                ATb = ps_at.tile([128, 512], F32, tag="at")
                mm(ATb[:, 0:256], ksT_sb[:, j0], qsT_sb[:, ci],
                   start=True, stop=True)
                Am0 = am0p.tile([128, 256], FM, tag="am0")
                nc.vector.tensor_mul(Am0[:, 0:128], ATb[:, 0:128], tri_sb[:])
                nc.scalar.copy(Am0[:, 128:256], ATb[:, 128:256])
                mm(ATb[:, 256:384], ksT_sb[:, j1], qsT_sb[:, ch],
                   start=True, stop=True)
                Am1 = am1p.tile([128, 128], FM, tag="am1")
                nc.vector.tensor_mul(Am1[:], ATb[:, 256:384], tri_sb[:])

                if pend is not None:
                    _emit_out(nc, mm, pend, hn_sb, qsT_sb, otT_sb, otT, ps_ot)
                pend = (m, S_prev, Am0, Am1)
                S_prev = S_new
            _emit_out(nc, mm, pend, hn_sb, qsT_sb, otT_sb, otT, ps_ot)

    _split_multi_waits(nc)
    _PROG[key] = nc
    return nc


_OUT_PIECES = {3: (0, 4), 7: (4, 8), 11: (8, 12), 13: (12, 14),
               14: (14, 15), 15: (15, 16)}


def _emit_out(nc, mm, pend, hn_sb, qsT_sb, otT_sb, otT, ps_ot):
    import concourse.mybir as mybir
    m, S_m, Am0, Am1 = pend
    c = m * C
    j0 = slice(c, c + 128)
    j1 = slice(c + 128, c + 256)
    ci = slice(c, c + 256)
    OT = ps_ot.tile([128, 256], mybir.dt.float32, tag="ot")
    mm(OT[:], hn_sb[:, j0], Am0[:], start=True, stop=False)
    mm(OT[:, 128:256], hn_sb[:, j1], Am1[:], start=False, stop=False)
    mm(OT[:], S_m[:], qsT_sb[:, ci], start=False, stop=True)
    nc.scalar.copy(otT_sb[:, ci], OT[:])
    if m in _OUT_PIECES:
        lo, hi = _OUT_PIECES[m]
        s = slice(lo * C, hi * C)
        nc.scalar.dma_start(otT[:, s], otT_sb[:, s])


def _host_prep(q_alpha, k, h_norm, gamma_vec, causal_mask):
    gamma = np.clip(np.asarray(gamma_vec, np.float64), 1e-8, None)
    log_g = np.log(gamma)
    i_loc = (np.arange(W) % C).astype(np.float64)
    Sq = np.exp(np.outer(i_loc, log_g))          # [W, R] gamma^(i%C)
    Skneg = np.exp(np.outer(-i_loc, log_g))      # gamma^-(j%C)
    Sk2 = np.exp(np.outer(C - i_loc, log_g))     # gamma^(C - j%C)
    g256 = np.exp(C * log_g).astype(np.float32).reshape(128, 1)

    tri = np.ascontiguousarray(np.asarray(causal_mask, np.float32).T)

    def blockify(x):  # [W, 128] -> [128, (blk, 128)]
        return np.ascontiguousarray(
            x.reshape(NBLK, 128, 128).transpose(1, 0, 2).reshape(128, W))

    in_maps = []
    for b in range(B):
        q64 = np.asarray(q_alpha[b], np.float64)
        k64 = np.asarray(k[b], np.float64)
        in_maps.append({
            "qsT": np.ascontiguousarray((q64 * Sq).T.astype(np.float32)),
            "ksT": np.ascontiguousarray((k64 * Skneg).T.astype(np.float32)),
            "k2n": blockify((k64 * Sk2).astype(np.float32)),
            "hn": blockify(np.ascontiguousarray(h_norm[b], np.float32)),
            "tri": tri,
            "g256": g256,
        })
    return in_maps


def _ensure_ntff_hook():
    try:
        from antenv import axon_hooks  # noqa: F401
        return
    except ImportError:
        pass
    import types
    import antenv
    try:
        import trn_agent_boot.trn_boot as tb
        hook = tb._ntff_profile_via_ctypes("/opt/axon/libaxon_pjrt.so")
    except Exception:
        hook = None
    mod = types.ModuleType("antenv.axon_hooks")
    mod.get_axon_ntff_profile_hook = lambda: hook
    mod.set_axon_ntff_profile_hook = lambda h: None
    sys.modules["antenv.axon_hooks"] = mod
    antenv.axon_hooks = mod


_last = {"exec_time_ns": None}


def kernel(q_alpha, k, h_norm, gamma_vec, causal_mask, decay_diff,
           _trace=False, _use_f32r=None):
    if _use_f32r is None:
        _use_f32r = os.environ.get("BD_F32R", "0") == "1"
    trace = _trace or os.environ.get("BD_TRACE", "0") == "1"
    from concourse.bass_utils import run_bass_kernel_spmd

    nc = _build_program(use_f32r=_use_f32r)
    in_maps = _host_prep(q_alpha, k, h_norm, gamma_vec, causal_mask)
    kwargs = {}
    if trace:
        _ensure_ntff_hook()
        import concourse.bass_utils as bu
        bu.upload_artifacts = lambda tmpdir: tmpdir  # no bucket in container
        kwargs = dict(trace=True, tmpdir=os.environ.get("BD_TRACE_DIR") or None)
    res = run_bass_kernel_spmd(nc, in_maps, list(range(B)), **kwargs)
    _last["exec_time_ns"] = res.exec_time_ns
    out = np.empty((B, W, D), np.float32)
    for b in range(B):
        out[b] = res.results[b]["otT"].T
    return out
